# revision 16
# baseline (speedup 1.0000x reference)
"""Trainium2 Bass kernel for an attention block with softmax over the QUERY axis.

Reference computation (per batch b):
    Q = x_b @ Wq^T + bq ; K = x_b @ Wk^T + bk ; V = x_b @ Wv^T + bv
    S = Q @ K^T / sqrt(256)
    attn = softmax(S, axis over queries)      # couples rows, not columns
    out_b = attn @ V

Algebraic facts exploited:
  * softmax over q removes any score term constant along q.  The query
    bias contributes  c[k] = bq . K[k]  — constant along q — so bq drops
    out of the output entirely.
  * S^T = K Q^T = (K Wq) x^T, and  K Wq = x Wk^T Wq + bk Wq,  so with the
    host-precomputed  W2 = Wk^T Wq  (256x256) and  gb = Wq^T bk  the score
    operand  G^T = W2^T x^T + gb  comes straight from x — neither the Q
    nor the K projection is ever materialized on device.

Sharding over 8 NeuronCores: core m handles batch b = m // 2 and the
key/value half h = m % 2.  The host passes x_b^T with the query axis
rotated so the core's 2048 keys are always columns 0:2048; each core holds
the full query range for its batch, so the softmax over queries is fully
local.  Each core produces a partial output (sum over its 2048 keys); the
host rotates back and adds the two partials per batch.  No collectives.

On-core dataflow (matmul operands bf16, fp32 accumulation):
    xT  (256d, 4096s)   host-transposed input, d on partitions
    GT  (256d, 2048k) = W2^T @ x_h^T + gb   (d on partitions)
    V   (2048k, 256e) = x_h @ Wv^T + bv     (natural layout, k on partitions)
    ST  (k, q) tiles  = GT^T slices @ xT    (scores transposed, PSUM)
    e   = exp(ST/16)  on ACT; row sums via the activation accumulator
    V'  = V / s_k     per-partition scale (in place, DVE)
    out = e^T @ V'    keys in rounds of 2 tiles, fp32 partials in SBUF

Schedule: one uniform loop over the 16 key tiles.  Each tile emits 4
quarter-row score groups + exp, then a block of PE filler: the V/GT
projections for k0/k1, and from k2 on the AV chains of the key pair
finished two tiles ago.  That keeps the PE dense at every tile while the
exp stream (whose cost per tile is just under the PE's) never gates.  AV
chains land 4-query-tiles-to-a-PSUM-tile so partial adds are 1024-wide
DVE ops; output leaves in 8 quarter-group DMAs.  Inputs are split across
many DMA queues (a single queue moves only ~70 GB/s).  20 warmup matmuls
bridge from queue start to data arrival (~14 us) so the HAM window
stays busy and real work issues at 2.4 GHz.
"""

import numpy as np
import ml_dtypes

import concourse.bass as bass
import concourse.tile as tile
from concourse import bacc, mybir
from concourse.bass_utils import run_bass_kernel_spmd

BF16 = ml_dtypes.bfloat16
F32 = mybir.dt.float32
BF = mybir.dt.bfloat16

B, S, D = 4, 4096, 256
NCORES = 8
KH = S // 2          # 2048 keys per core
NKT = KH // 128      # 16 key tiles
NQT = S // 128       # 32 query tiles

EXP = mybir.ActivationFunctionType.Exp
IDENT = mybir.ActivationFunctionType.Identity
AX = mybir.AxisListType.X
ADD = mybir.AluOpType.add


def _emit(tc, xT, wv2, gbc, bvr, out):
    nc = tc.nc

    with tc.tile_pool(name="const", bufs=1) as cpool, \
         tc.tile_pool(name="big", bufs=1) as bpool, \
         tc.tile_pool(name="work", bufs=4) as wpool:

        w_sb = cpool.tile([128, 2, 2 * D], BF, name="wsb", tag="wsb")
        gb_sb = cpool.tile([128, 2, 1], F32, name="gbsb", tag="gbsb")
        bv_sb = cpool.tile([1, D], BF, name="bv", tag="bv")
        ones = cpool.tile([1, 128], BF, name="ones", tag="ones")
        nc.vector.memset(ones, 1.0)
        junk = cpool.tile([128, 512], BF, name="junk", tag="junk")
        nc.vector.memset(junk, 1.0)

        xT_sb = cpool.tile([128, 2 * S], BF, name="xTsb", tag="xTsb")
        xt_out = xT_sb.rearrange("p (t s) -> p t s", t=2)
        xt_in = xT.rearrange("(t p) s -> p t s", p=128)
        w_in = wv2.rearrange("(t p) c -> p t c", p=128)

        # The DMA engines round-robin across all active queues, so every
        # stream completes together — the first-needed chunks must run
        # ALONE to arrive early.  Later chunks are held back by a WAW dep:
        # a 1-column memset into each landing zone that itself queues (on
        # the DVE) behind a probe read of the last first-wave chunk.
        nc.sync.dma_start(w_sb[:, :, 0:D], w_in[:, :, 0:D])          # W2^T
        for c in range(4):
            nc.sync.dma_start(xt_out[:, :, 256 * c:256 * (c + 1)],
                              xt_in[:, :, 256 * c:256 * (c + 1)])
        nc.sync.dma_start(w_sb[:, :, D:2 * D], w_in[:, :, D:2 * D])  # Wv^T
        nc.sync.dma_start(gb_sb, gbc.rearrange("(t p) c -> p t c", p=128))
        nc.sync.dma_start(bv_sb, bvr)
        for c in range(2):
            nc.sync.dma_start(xt_out[:, :, 1024 + 512 * c:1024 + 512 * (c + 1)],
                              xt_in[:, :, 1024 + 512 * c:1024 + 512 * (c + 1)])
        for c in range(4):
            nc.sync.dma_start(xt_out[:, :, KH + 512 * c:KH + 512 * (c + 1)],
                              xt_in[:, :, KH + 512 * c:KH + 512 * (c + 1)])
        xTs = [xT_sb[:, 0:S], xT_sb[:, S:2 * S]]

        def w2(i):                       # W2^T rows [128i,128(i+1))
            return w_sb[:, i, 0:D]

        def wv(i):
            return w_sb[:, i, D:2 * D]

        # ---- persistent tiles ----
        GT_sb = [bpool.tile([128, KH], BF, name=f"GT{i}", tag=f"GT{i}")
                 for i in range(2)]
        Vb_sb = [bpool.tile([128, D], BF, name=f"Vb{k}", tag=f"Vb{k}")
                 for k in range(NKT)]
        e_sb = [bpool.tile([128, S], BF, name=f"e{k}", tag=f"e{k}")
                for k in range(NKT)]
        # fp32 partial sums, 8 query tiles per physical tile so partial
        # adds are 1024-wide DVE ops and stores are 4-query-tile DMAs
        part_sb = [bpool.tile([128, 8, D], F32, name=f"pt{g}", tag=f"pt{g}")
                   for g in range(4)]
        bvb_sb = cpool.tile([128, D], BF, name="bvb", tag="bvb")
        out_r = out.rearrange("(j p) c -> p j c", p=128)

        def part(j0, width):
            g, jj = divmod(j0, 8)
            return part_sb[g][:, jj:jj + width, :]

        # ---- emitters ----
        def gt_group(sub, i, kb):
            cs = slice(512 * kb, 512 * (kb + 1))
            nc.tensor.matmul(sub, w2(0)[:, 128 * i:128 * (i + 1)],
                             xTs[0][:, cs], start=True, stop=False)
            nc.tensor.matmul(sub, w2(1)[:, 128 * i:128 * (i + 1)],
                             xTs[1][:, cs], start=False, stop=True)
            nc.vector.tensor_scalar_add(GT_sb[i][:, cs], sub,
                                        gb_sb[:, i, 0:1])

        def v_group(sub, k):
            ks = slice(128 * k, 128 * (k + 1))
            nc.tensor.matmul(sub, xTs[0][:, ks], wv(0), start=True, stop=False)
            nc.tensor.matmul(sub, xTs[1][:, ks], wv(1), start=False, stop=True)
            nc.vector.tensor_tensor(Vb_sb[k], sub, bvb_sb, op=ADD)

        def emit_score_mms(sub, k, q0):
            qs = slice(q0, q0 + 512)
            nc.tensor.matmul(sub, GT_sb[0][:, 128 * k:128 * (k + 1)],
                             xTs[0][:, qs], start=True, stop=False)
            nc.tensor.matmul(sub, GT_sb[1][:, 128 * k:128 * (k + 1)],
                             xTs[1][:, qs], start=False, stop=True)

        def emit_row_scale(k, sparts):
            ssum = wpool.tile([128, 1], F32, name="ssum", tag="ssum")
            nc.vector.reduce_sum(ssum, sparts, axis=AX)
            rs = wpool.tile([128, 1], F32, name="rs", tag="rs")
            nc.vector.reciprocal(rs, ssum)
            nc.vector.tensor_scalar_mul(Vb_sb[k], Vb_sb[k], rs)

        def emit_av_group(pa, j0, pair):
            # 4 query tiles j0..j0+3 over key tiles {2*pair, 2*pair+1}
            for jj in range(4):
                sub = pa[:, D * jj:D * (jj + 1)]
                for n in range(2):
                    k = 2 * pair + n
                    nc.tensor.matmul(sub,
                                     e_sb[k][:, 128 * (j0 + jj):
                                             128 * (j0 + jj + 1)],
                                     Vb_sb[k], start=(n == 0), stop=(n == 1))
            dst = part(j0, 4)
            if pair == 0:
                nc.vector.tensor_copy(dst, pa)
            else:
                nc.vector.tensor_tensor(dst, pa, dst, op=ADD)
            if pair == 7:                # final round: store 4 query tiles
                nc.sync.dma_start(out_r[:, j0:j0 + 4, :], part(j0, 4))

        # ====== prologue: warmup + bias + first GT block ==================
        with tc.tile_pool(name="ps0", bufs=4, space="PSUM") as ps0:
            def slot0():
                return ps0.tile([128, 512], F32, name="ps0t", tag="ps0t")

            warm = slot0()
            for _ in range(22):
                nc.tensor.matmul(warm, junk[:, 0:128], junk,
                                 start=True, stop=True)
            pt = slot0()
            nc.tensor.matmul(pt[:, 0:D], ones, bv_sb, start=True, stop=True)
            nc.vector.tensor_copy(bvb_sb, pt[:, 0:D])
            for i in range(2):
                gt_group(slot0(), i, 0)

        # ====== main loop: scores + exp + filler/AV per key tile ==========
        with tc.tile_pool(name="psav", bufs=2, space="PSUM") as psav:

            def filler_block(k):
                if k == 0:
                    for v in range(8):
                        pa = psav.tile([128, 1024], F32, name="psavt",
                                       tag="psavt")
                        v_group(pa[:, 0:D], v)
                        if v == 1:
                            gt_group(pa[:, 512:1024], 0, 1)
                        elif v == 3:
                            gt_group(pa[:, 512:1024], 1, 1)
                elif k == 1:
                    for v in range(8, NKT):
                        pa = psav.tile([128, 1024], F32, name="psavt",
                                       tag="psavt")
                        v_group(pa[:, 0:D], v)
                        if v < 12:
                            gt_group(pa[:, 512:1024], (v - 8) % 2,
                                     2 + (v - 8) // 2)
                else:
                    pair, half = k // 2 - 1, k % 2
                    for j0 in range(16 * half, 16 * half + 16, 4):
                        pa = psav.tile([128, 1024], F32, name="psavt",
                                       tag="psavt")
                        emit_av_group(pa, j0, pair)

            with tc.tile_pool(name="psb", bufs=2, space="PSUM") as psb:
                for k in range(NKT):
                    sparts = wpool.tile([128, 4], F32, name="sparts4",
                                        tag="sparts4")
                    for quarter in range(4):
                        s = psb.tile([128, 1024], F32, name="psbt",
                                     tag="psbt")
                        for g in range(2):
                            emit_score_mms(s[:, 512 * g:512 * (g + 1)], k,
                                           1024 * quarter + 512 * g)
                        nc.scalar.activation(
                            e_sb[k][:, 1024 * quarter:1024 * (quarter + 1)],
                            s, EXP, scale=1.0 / 16.0,
                            accum_out=sparts[:, quarter:quarter + 1])
                        if k == 0 and quarter == 1:
                            filler_block(0)  # cover the xT tail-chunk DMA
                    if k != 0:
                        filler_block(k)
                    emit_row_scale(k, sparts)

            # ====== tail: AV pair 7 + partial add + store =================
            with tc.tile_pool(name="psav2", bufs=2, space="PSUM") as psav2:
                for j0 in range(0, NQT, 4):
                    if j0 % 8 == 0:
                        pa = psav.tile([128, 1024], F32, name="psavt",
                                       tag="psavt")
                    else:
                        pa = psav2.tile([128, 1024], F32, name="psav2t",
                                        tag="psav2t")
                    emit_av_group(pa, j0, 7)


def build():
    nc = bacc.Bacc("TRN2", target_bir_lowering=False, debug=False)
    xT = nc.dram_tensor("xT", [D, S], BF, kind="ExternalInput").ap()
    wv2 = nc.dram_tensor("wv2", [D, 2 * D], BF, kind="ExternalInput").ap()
    gbc = nc.dram_tensor("gbc", [D, 1], F32, kind="ExternalInput").ap()
    bvr = nc.dram_tensor("bvr", [1, D], BF, kind="ExternalInput").ap()
    out = nc.dram_tensor("out", [S, D], F32, kind="ExternalOutput").ap()

    with tile.TileContext(nc) as tc:
        _emit(tc, xT, wv2, gbc, bvr, out)
    nc.compile()
    return nc


_NC = None


def _get_nc():
    global _NC
    if _NC is None:
        _NC = build()
    return _NC


def make_in_maps(x, Wq, bq, Wk, bk, Wv, bv):
    # bq cancels under the softmax-over-queries (see module docstring)
    w2T = (np.asarray(Wk, np.float32).T @ np.asarray(Wq, np.float32))
    wv2 = np.ascontiguousarray(
        np.concatenate([w2T, np.asarray(Wv).T], axis=1)).astype(BF16)
    gbc = (np.asarray(Wq, np.float32).T
           @ np.asarray(bk, np.float32)).reshape(D, 1).astype(np.float32)
    bvr = np.asarray(bv).reshape(1, D).astype(BF16)
    in_maps = []
    for core in range(NCORES):
        b, h = divmod(core, 2)
        xTb = np.asarray(x[b]).T.astype(BF16)
        if h:  # rotate so this core's keys are always columns 0:KH
            xTb = np.concatenate([xTb[:, KH:], xTb[:, :KH]], axis=1)
        in_maps.append({
            "xT": np.ascontiguousarray(xTb),
            "wv2": wv2, "gbc": gbc, "bvr": bvr,
        })
    return in_maps


def run(x, Wq, bq, Wk, bk, Wv, bv, trace=False):
    """Run on the 8 cores; returns (full_output, BassKernelResults)."""
    nc = _get_nc()
    in_maps = make_in_maps(x, Wq, bq, Wk, bk, Wv, bv)
    res = run_bass_kernel_spmd(nc, in_maps, core_ids=list(range(NCORES)),
                               trace=trace)
    parts = []
    for core in range(NCORES):
        p = res.results[core]["out"]
        if core % 2:  # undo the query rotation
            p = np.concatenate([p[KH:], p[:KH]], axis=0)
        parts.append(p)
    full = np.stack([parts[2 * b] + parts[2 * b + 1] for b in range(B)], axis=0)
    return full.astype(np.float32), res


def kernel(x, Wq, bq, Wk, bk, Wv, bv):
    full, _ = run(x, Wq, bq, Wk, bk, Wv, bv, trace=False)
    return full


# revision 18
# speedup vs baseline: 1.1962x; 1.1962x over previous
"""Trainium2 Bass kernel for an attention block with softmax over the QUERY axis.

Reference computation (per batch b):
    Q = x_b @ Wq^T + bq ; K = x_b @ Wk^T + bk ; V = x_b @ Wv^T + bv
    S = Q @ K^T / sqrt(256)
    attn = softmax(S, axis over queries)      # couples rows, not columns
    out_b = attn @ V

Algebraic facts exploited:
  * softmax over q removes any score term constant along q.  The query
    bias contributes  c[k] = bq . K[k]  — constant along q — so bq drops
    out of the output entirely.
  * S^T = K Q^T = (K Wq) x^T, and  K Wq = x Wk^T Wq + bk Wq,  so with the
    host-precomputed  W2 = Wk^T Wq  (256x256) and  gb = Wq^T bk  the score
    operand  G^T = W2^T x^T + gb  comes straight from x — neither the Q
    nor the K projection is ever materialized on device.

Sharding over 8 NeuronCores: core m handles batch b = m // 2 and the
key/value half h = m % 2.  The host passes x_b^T with the query axis
rotated so the core's 2048 keys are always columns 0:2048; each core holds
the full query range for its batch, so the softmax over queries is fully
local.  Each core produces a partial output (sum over its 2048 keys); the
host rotates back and adds the two partials per batch.  No collectives.

On-core dataflow (matmul operands bf16, fp32 accumulation):
    xT  (256d, 4096s)   host-transposed input, d on partitions
    GT  (256d, 2048k) = W2^T @ x_h^T + gb   (d on partitions)
    V   (2048k, 256e) = x_h @ Wv^T + bv     (natural layout, k on partitions)
    ST  (k, q) tiles  = GT^T slices @ xT    (scores transposed, PSUM)
    e   = exp(ST/16)  on ACT; row sums via the activation accumulator
    V'  = V / s_k     per-partition scale (in place, DVE)
    out = e^T @ V'    keys in rounds of 2 tiles, fp32 partials in SBUF

Schedule: one uniform loop over the 16 key tiles.  Each tile emits 4
quarter-row score groups + exp, then a block of PE filler: the V/GT
projections for k0/k1, and from k2 on the AV chains of the key pair
finished two tiles ago.  That keeps the PE dense at every tile while the
exp stream (whose cost per tile is just under the PE's) never gates.  AV
chains land 4-query-tiles-to-a-PSUM-tile so partial adds are 1024-wide
DVE ops; output leaves in 8 quarter-group DMAs.  Inputs are split across
many DMA queues (a single queue moves only ~70 GB/s).  20 warmup matmuls
bridge from queue start to data arrival (~14 us) so the HAM window
stays busy and real work issues at 2.4 GHz.
"""

import numpy as np
import ml_dtypes

import concourse.bass as bass
import concourse.tile as tile
from concourse import bacc, mybir
from concourse.bass_utils import run_bass_kernel_spmd

BF16 = ml_dtypes.bfloat16
F32 = mybir.dt.float32
BF = mybir.dt.bfloat16

B, S, D = 4, 4096, 256
NCORES = 8
KH = S // 2          # 2048 keys per core
NKT = KH // 128      # 16 key tiles
NQT = S // 128       # 32 query tiles

EXP = mybir.ActivationFunctionType.Exp
IDENT = mybir.ActivationFunctionType.Identity
AX = mybir.AxisListType.X
ADD = mybir.AluOpType.add


def _emit(tc, xT, wv2, gbc, bvr, out):
    nc = tc.nc

    with tc.tile_pool(name="const", bufs=1) as cpool, \
         tc.tile_pool(name="big", bufs=1) as bpool, \
         tc.tile_pool(name="work", bufs=4) as wpool:

        w_sb = cpool.tile([128, 2, 2 * D], BF, name="wsb", tag="wsb")
        gb_sb = cpool.tile([128, 2, 1], F32, name="gbsb", tag="gbsb")
        bv_sb = cpool.tile([1, D], BF, name="bv", tag="bv")
        ones = cpool.tile([1, 128], BF, name="ones", tag="ones")
        nc.vector.memset(ones, 1.0)
        junk = cpool.tile([128, 512], BF, name="junk", tag="junk")
        nc.vector.memset(junk, 1.0)

        xT_sb = cpool.tile([128, 2 * S], BF, name="xTsb", tag="xTsb")
        xt_out = xT_sb.rearrange("p (t s) -> p t s", t=2)
        xt_in = xT.rearrange("(t p) s -> p t s", p=128)
        w_in = wv2.rearrange("(t p) c -> p t c", p=128)

        # The DMA engines round-robin across all active queues, so every
        # stream completes together — the first-needed chunks must run
        # ALONE to arrive early.  Later chunks are held back by a WAW dep:
        # a 1-column memset into each landing zone that itself queues (on
        # the DVE) behind a probe read of the last first-wave chunk.
        nc.sync.dma_start(w_sb[:, :, 0:D], w_in[:, :, 0:D])          # W2^T
        for c in range(4):
            nc.sync.dma_start(xt_out[:, :, 256 * c:256 * (c + 1)],
                              xt_in[:, :, 256 * c:256 * (c + 1)])
        nc.sync.dma_start(w_sb[:, :, D:2 * D], w_in[:, :, D:2 * D])  # Wv^T
        nc.sync.dma_start(gb_sb, gbc.rearrange("(t p) c -> p t c", p=128))
        nc.sync.dma_start(bv_sb, bvr)
        for c in range(2):
            nc.sync.dma_start(xt_out[:, :, 1024 + 512 * c:1024 + 512 * (c + 1)],
                              xt_in[:, :, 1024 + 512 * c:1024 + 512 * (c + 1)])
        for c in range(4):
            nc.sync.dma_start(xt_out[:, :, KH + 512 * c:KH + 512 * (c + 1)],
                              xt_in[:, :, KH + 512 * c:KH + 512 * (c + 1)])
        xTs = [xT_sb[:, 0:S], xT_sb[:, S:2 * S]]

        def w2(i):                       # W2^T rows [128i,128(i+1))
            return w_sb[:, i, 0:D]

        def wv(i):
            return w_sb[:, i, D:2 * D]

        # ---- persistent tiles ----
        GT_sb = [bpool.tile([128, KH], BF, name=f"GT{i}", tag=f"GT{i}")
                 for i in range(2)]
        Vb_sb = [bpool.tile([128, D], BF, name=f"Vb{k}", tag=f"Vb{k}")
                 for k in range(NKT)]
        e_sb = [bpool.tile([128, S], BF, name=f"e{k}", tag=f"e{k}")
                for k in range(NKT)]
        # fp32 partial sums, 8 query tiles per physical tile so partial
        # adds are 1024-wide DVE ops and stores are 4-query-tile DMAs
        part_sb = [bpool.tile([128, 8, D], F32, name=f"pt{g}", tag=f"pt{g}")
                   for g in range(4)]
        ob_sb = [bpool.tile([128, 4, D], BF, name=f"ob{g}", tag=f"ob{g}")
                 for g in range(4)]
        bvb_sb = cpool.tile([128, D], BF, name="bvb", tag="bvb")
        out_r = out.rearrange("(j p) c -> p j c", p=128)

        def part(j0, width):
            g, jj = divmod(j0, 8)
            return part_sb[g][:, jj:jj + width, :]

        # ---- emitters ----
        def gt_group(sub, i, kb):
            cs = slice(512 * kb, 512 * (kb + 1))
            nc.tensor.matmul(sub, w2(0)[:, 128 * i:128 * (i + 1)],
                             xTs[0][:, cs], start=True, stop=False)
            nc.tensor.matmul(sub, w2(1)[:, 128 * i:128 * (i + 1)],
                             xTs[1][:, cs], start=False, stop=True)
            nc.vector.tensor_scalar_add(GT_sb[i][:, cs], sub,
                                        gb_sb[:, i, 0:1])

        def v_group(sub, k):
            ks = slice(128 * k, 128 * (k + 1))
            nc.tensor.matmul(sub, xTs[0][:, ks], wv(0), start=True, stop=False)
            nc.tensor.matmul(sub, xTs[1][:, ks], wv(1), start=False, stop=True)
            nc.vector.tensor_tensor(Vb_sb[k], sub, bvb_sb, op=ADD)

        def emit_score_mms(sub, k, q0):
            qs = slice(q0, q0 + 512)
            nc.tensor.matmul(sub, GT_sb[0][:, 128 * k:128 * (k + 1)],
                             xTs[0][:, qs], start=True, stop=False)
            nc.tensor.matmul(sub, GT_sb[1][:, 128 * k:128 * (k + 1)],
                             xTs[1][:, qs], start=False, stop=True)

        def emit_row_scale(k, sparts):
            ssum = wpool.tile([128, 1], F32, name="ssum", tag="ssum")
            nc.vector.reduce_sum(ssum, sparts, axis=AX)
            rs = wpool.tile([128, 1], F32, name="rs", tag="rs")
            nc.vector.reciprocal(rs, ssum)
            nc.vector.tensor_scalar_mul(Vb_sb[k], Vb_sb[k], rs)

        def emit_av_group(pa, j0, pair):
            # 4 query tiles j0..j0+3 over key tiles {2*pair, 2*pair+1}
            for jj in range(4):
                sub = pa[:, D * jj:D * (jj + 1)]
                for n in range(2):
                    k = 2 * pair + n
                    nc.tensor.matmul(sub,
                                     e_sb[k][:, 128 * (j0 + jj):
                                             128 * (j0 + jj + 1)],
                                     Vb_sb[k], start=(n == 0), stop=(n == 1))
            dst = part(j0, 4)
            if pair == 0:
                nc.vector.tensor_copy(dst, pa)
            elif pair < 7:
                nc.vector.tensor_tensor(dst, pa, dst, op=ADD)
            else:                        # final round: bf16 store, host adds
                ob = ob_sb[(j0 // 4) % 4]
                nc.vector.tensor_tensor(ob, pa, dst, op=ADD)
                nc.sync.dma_start(out_r[:, j0:j0 + 4, :], ob)

        # ====== prologue: warmup + bias + first GT block ==================
        with tc.tile_pool(name="ps0", bufs=4, space="PSUM") as ps0:
            def slot0():
                return ps0.tile([128, 512], F32, name="ps0t", tag="ps0t")

            warm = slot0()
            for _ in range(8):
                nc.tensor.matmul(warm, junk[:, 0:128], junk,
                                 start=True, stop=True)
            pt = slot0()
            nc.tensor.matmul(pt[:, 0:D], ones, bv_sb, start=True, stop=True)
            nc.vector.tensor_copy(bvb_sb, pt[:, 0:D])
            for i in range(2):
                gt_group(slot0(), i, 0)

        # ====== main loop: scores + exp + filler/AV per key tile ==========
        with tc.tile_pool(name="psav", bufs=2, space="PSUM") as psav:

            def filler_block(k):
                if k == 0:
                    for v in range(8):
                        pa = psav.tile([128, 1024], F32, name="psavt",
                                       tag="psavt")
                        v_group(pa[:, 0:D], v)
                        if v == 1:
                            gt_group(pa[:, 512:1024], 0, 1)
                        elif v == 3:
                            gt_group(pa[:, 512:1024], 1, 1)
                elif k == 1:
                    for v in range(8, NKT):
                        pa = psav.tile([128, 1024], F32, name="psavt",
                                       tag="psavt")
                        v_group(pa[:, 0:D], v)
                        if v < 12:
                            gt_group(pa[:, 512:1024], (v - 8) % 2,
                                     2 + (v - 8) // 2)
                else:
                    pair, half = k // 2 - 1, k % 2
                    for j0 in range(16 * half, 16 * half + 16, 4):
                        pa = psav.tile([128, 1024], F32, name="psavt",
                                       tag="psavt")
                        emit_av_group(pa, j0, pair)

            with tc.tile_pool(name="psb", bufs=2, space="PSUM") as psb:
                for k in range(NKT):
                    sparts = wpool.tile([128, 4], F32, name="sparts4",
                                        tag="sparts4")
                    for quarter in range(4):
                        s = psb.tile([128, 1024], F32, name="psbt",
                                     tag="psbt")
                        for g in range(2):
                            emit_score_mms(s[:, 512 * g:512 * (g + 1)], k,
                                           1024 * quarter + 512 * g)
                        nc.scalar.activation(
                            e_sb[k][:, 1024 * quarter:1024 * (quarter + 1)],
                            s, EXP, scale=1.0 / 16.0,
                            accum_out=sparts[:, quarter:quarter + 1])
                        if k == 0 and quarter == 1:
                            filler_block(0)  # cover the xT tail-chunk DMA
                    if k != 0:
                        filler_block(k)
                    emit_row_scale(k, sparts)

            # ====== tail: AV pair 7 + partial add + store =================
            with tc.tile_pool(name="psav2", bufs=2, space="PSUM") as psav2:
                for j0 in range(0, NQT, 4):
                    if j0 % 8 == 0:
                        pa = psav.tile([128, 1024], F32, name="psavt",
                                       tag="psavt")
                    else:
                        pa = psav2.tile([128, 1024], F32, name="psav2t",
                                        tag="psav2t")
                    emit_av_group(pa, j0, 7)


def build():
    nc = bacc.Bacc("TRN2", target_bir_lowering=False, debug=False)
    xT = nc.dram_tensor("xT", [D, S], BF, kind="ExternalInput").ap()
    wv2 = nc.dram_tensor("wv2", [D, 2 * D], BF, kind="ExternalInput").ap()
    gbc = nc.dram_tensor("gbc", [D, 1], F32, kind="ExternalInput").ap()
    bvr = nc.dram_tensor("bvr", [1, D], BF, kind="ExternalInput").ap()
    out = nc.dram_tensor("out", [S, D], BF, kind="ExternalOutput").ap()

    with tile.TileContext(nc) as tc:
        _emit(tc, xT, wv2, gbc, bvr, out)
    nc.compile()
    return nc


_NC = None


def _get_nc():
    global _NC
    if _NC is None:
        _NC = build()
    return _NC


def make_in_maps(x, Wq, bq, Wk, bk, Wv, bv):
    # bq cancels under the softmax-over-queries (see module docstring)
    w2T = (np.asarray(Wk, np.float32).T @ np.asarray(Wq, np.float32))
    wv2 = np.ascontiguousarray(
        np.concatenate([w2T, np.asarray(Wv).T], axis=1)).astype(BF16)
    gbc = (np.asarray(Wq, np.float32).T
           @ np.asarray(bk, np.float32)).reshape(D, 1).astype(np.float32)
    bvr = np.asarray(bv).reshape(1, D).astype(BF16)
    in_maps = []
    for core in range(NCORES):
        b, h = divmod(core, 2)
        xTb = np.asarray(x[b]).T.astype(BF16)
        if h:  # rotate so this core's keys are always columns 0:KH
            xTb = np.concatenate([xTb[:, KH:], xTb[:, :KH]], axis=1)
        in_maps.append({
            "xT": np.ascontiguousarray(xTb),
            "wv2": wv2, "gbc": gbc, "bvr": bvr,
        })
    return in_maps


def run(x, Wq, bq, Wk, bk, Wv, bv, trace=False):
    """Run on the 8 cores; returns (full_output, BassKernelResults)."""
    nc = _get_nc()
    in_maps = make_in_maps(x, Wq, bq, Wk, bk, Wv, bv)
    res = run_bass_kernel_spmd(nc, in_maps, core_ids=list(range(NCORES)),
                               trace=trace)
    parts = []
    for core in range(NCORES):
        p = res.results[core]["out"].astype(np.float32)
        if core % 2:  # undo the query rotation
            p = np.concatenate([p[KH:], p[:KH]], axis=0)
        parts.append(p)
    full = np.stack([parts[2 * b] + parts[2 * b + 1] for b in range(B)], axis=0)
    return full.astype(np.float32), res


def kernel(x, Wq, bq, Wk, bk, Wv, bv):
    full, _ = run(x, Wq, bq, Wk, bk, Wv, bv, trace=False)
    return full


# revision 19
# speedup vs baseline: 1.2081x; 1.0099x over previous
"""Trainium2 Bass kernel for an attention block with softmax over the QUERY axis.

Reference computation (per batch b):
    Q = x_b @ Wq^T + bq ; K = x_b @ Wk^T + bk ; V = x_b @ Wv^T + bv
    S = Q @ K^T / sqrt(256)
    attn = softmax(S, axis over queries)      # couples rows, not columns
    out_b = attn @ V

Algebraic facts exploited:
  * softmax over q removes any score term constant along q.  The query
    bias contributes  c[k] = bq . K[k]  — constant along q — so bq drops
    out of the output entirely.
  * S^T = K Q^T = (K Wq) x^T, and  K Wq = x Wk^T Wq + bk Wq,  so with the
    host-precomputed  W2 = Wk^T Wq  (256x256) and  gb = Wq^T bk  the score
    operand  G^T = W2^T x^T + gb  comes straight from x — neither the Q
    nor the K projection is ever materialized on device.

Sharding over 8 NeuronCores: core m handles batch b = m // 2 and the
key/value half h = m % 2.  The host passes x_b^T with the query axis
rotated so the core's 2048 keys are always columns 0:2048; each core holds
the full query range for its batch, so the softmax over queries is fully
local.  Each core produces a partial output (sum over its 2048 keys); the
host rotates back and adds the two partials per batch.  No collectives.

On-core dataflow (matmul operands bf16, fp32 accumulation):
    xT  (256d, 4096s)   host-transposed input, d on partitions
    GT  (256d, 2048k) = W2^T @ x_h^T + gb   (d on partitions)
    V   (2048k, 256e) = x_h @ Wv^T + bv     (natural layout, k on partitions)
    ST  (k, q) tiles  = GT^T slices @ xT    (scores transposed, PSUM)
    e   = exp(ST/16)  on ACT; row sums via the activation accumulator
    V'  = V / s_k     per-partition scale (in place, DVE)
    out = e^T @ V'    keys in rounds of 2 tiles, fp32 partials in SBUF

Schedule: one uniform loop over the 16 key tiles.  Each tile emits 4
quarter-row score groups + exp, then a block of PE filler: the V/GT
projections for k0/k1, and from k2 on the AV chains of the key pair
finished two tiles ago.  That keeps the PE dense at every tile while the
exp stream (whose cost per tile is just under the PE's) never gates.  AV
chains land 4-query-tiles-to-a-PSUM-tile so partial adds are 1024-wide
DVE ops; output leaves in 8 quarter-group DMAs.  Inputs are split across
many DMA queues (a single queue moves only ~70 GB/s).  20 warmup matmuls
bridge from queue start to data arrival (~14 us) so the HAM window
stays busy and real work issues at 2.4 GHz.
"""

import numpy as np
import ml_dtypes

import concourse.bass as bass
import concourse.tile as tile
from concourse import bacc, mybir
from concourse.bass_utils import run_bass_kernel_spmd

BF16 = ml_dtypes.bfloat16
F32 = mybir.dt.float32
BF = mybir.dt.bfloat16

B, S, D = 4, 4096, 256
NCORES = 8
KH = S // 2          # 2048 keys per core
NKT = KH // 128      # 16 key tiles
NQT = S // 128       # 32 query tiles

EXP = mybir.ActivationFunctionType.Exp
IDENT = mybir.ActivationFunctionType.Identity
AX = mybir.AxisListType.X
ADD = mybir.AluOpType.add


def _emit(tc, xT, wv2, gbc, bvr, out):
    nc = tc.nc

    with tc.tile_pool(name="const", bufs=1) as cpool, \
         tc.tile_pool(name="big", bufs=1) as bpool, \
         tc.tile_pool(name="work", bufs=4) as wpool:

        w_sb = cpool.tile([128, 2, 2 * D], BF, name="wsb", tag="wsb")
        gb_sb = cpool.tile([128, 2, 1], F32, name="gbsb", tag="gbsb")
        bv_sb = cpool.tile([1, D], BF, name="bv", tag="bv")
        ones = cpool.tile([1, 128], BF, name="ones", tag="ones")
        nc.vector.memset(ones, 1.0)
        junk = cpool.tile([128, 512], BF, name="junk", tag="junk")
        nc.vector.memset(junk, 1.0)

        xT_sb = cpool.tile([128, 2 * S], BF, name="xTsb", tag="xTsb")
        xt_out = xT_sb.rearrange("p (t s) -> p t s", t=2)
        xt_in = xT.rearrange("(t p) s -> p t s", p=128)
        w_in = wv2.rearrange("(t p) c -> p t c", p=128)

        # The DMA engines round-robin across all active queues, so every
        # stream completes together — the first-needed chunks must run
        # ALONE to arrive early.  Later chunks are held back by a WAW dep:
        # a 1-column memset into each landing zone that itself queues (on
        # the DVE) behind a probe read of the last first-wave chunk.
        nc.sync.dma_start(w_sb[:, :, 0:D], w_in[:, :, 0:D])          # W2^T
        for c in range(4):
            nc.sync.dma_start(xt_out[:, :, 256 * c:256 * (c + 1)],
                              xt_in[:, :, 256 * c:256 * (c + 1)])
        nc.sync.dma_start(w_sb[:, :, D:2 * D], w_in[:, :, D:2 * D])  # Wv^T
        nc.sync.dma_start(gb_sb, gbc.rearrange("(t p) c -> p t c", p=128))
        nc.sync.dma_start(bv_sb, bvr)
        for c in range(2):
            nc.sync.dma_start(xt_out[:, :, 1024 + 512 * c:1024 + 512 * (c + 1)],
                              xt_in[:, :, 1024 + 512 * c:1024 + 512 * (c + 1)])
        for c in range(4):
            nc.sync.dma_start(xt_out[:, :, KH + 512 * c:KH + 512 * (c + 1)],
                              xt_in[:, :, KH + 512 * c:KH + 512 * (c + 1)])
        xTs = [xT_sb[:, 0:S], xT_sb[:, S:2 * S]]

        def w2(i):                       # W2^T rows [128i,128(i+1))
            return w_sb[:, i, 0:D]

        def wv(i):
            return w_sb[:, i, D:2 * D]

        # ---- persistent tiles ----
        GT_sb = [bpool.tile([128, KH], BF, name=f"GT{i}", tag=f"GT{i}")
                 for i in range(2)]
        Vb_sb = [bpool.tile([128, D], BF, name=f"Vb{k}", tag=f"Vb{k}")
                 for k in range(NKT)]
        e_sb = [bpool.tile([128, S], BF, name=f"e{k}", tag=f"e{k}")
                for k in range(NKT)]
        # fp32 partial sums, 8 query tiles per physical tile so partial
        # adds are 1024-wide DVE ops and stores are 4-query-tile DMAs
        part_sb = [bpool.tile([128, 8, D], F32, name=f"pt{g}", tag=f"pt{g}")
                   for g in range(4)]
        ob_sb = [bpool.tile([128, 4, D], BF, name=f"ob{g}", tag=f"ob{g}")
                 for g in range(4)]
        bvb_sb = cpool.tile([128, D], BF, name="bvb", tag="bvb")
        out_r = out.rearrange("(j p) c -> p j c", p=128)

        def part(j0, width):
            g, jj = divmod(j0, 8)
            return part_sb[g][:, jj:jj + width, :]

        # ---- emitters ----
        def gt_group(sub, i, kb):
            cs = slice(512 * kb, 512 * (kb + 1))
            nc.tensor.matmul(sub, w2(0)[:, 128 * i:128 * (i + 1)],
                             xTs[0][:, cs], start=True, stop=False)
            nc.tensor.matmul(sub, w2(1)[:, 128 * i:128 * (i + 1)],
                             xTs[1][:, cs], start=False, stop=True)
            nc.vector.tensor_scalar_add(GT_sb[i][:, cs], sub,
                                        gb_sb[:, i, 0:1])

        def v_group(sub, k):
            ks = slice(128 * k, 128 * (k + 1))
            nc.tensor.matmul(sub, xTs[0][:, ks], wv(0), start=True, stop=False)
            nc.tensor.matmul(sub, xTs[1][:, ks], wv(1), start=False, stop=True)
            nc.vector.tensor_tensor(Vb_sb[k], sub, bvb_sb, op=ADD)

        def emit_score_mms(sub, k, q0):
            qs = slice(q0, q0 + 512)
            nc.tensor.matmul(sub, GT_sb[0][:, 128 * k:128 * (k + 1)],
                             xTs[0][:, qs], start=True, stop=False)
            nc.tensor.matmul(sub, GT_sb[1][:, 128 * k:128 * (k + 1)],
                             xTs[1][:, qs], start=False, stop=True)

        def emit_row_scale(k, sparts):
            ssum = wpool.tile([128, 1], F32, name="ssum", tag="ssum")
            nc.vector.reduce_sum(ssum, sparts, axis=AX)
            rs = wpool.tile([128, 1], F32, name="rs", tag="rs")
            nc.vector.reciprocal(rs, ssum)
            nc.vector.tensor_scalar_mul(Vb_sb[k], Vb_sb[k], rs)

        def emit_av_group(pa, j0, pair):
            # 4 query tiles j0..j0+3 over key tiles {2*pair, 2*pair+1}
            for jj in range(4):
                sub = pa[:, D * jj:D * (jj + 1)]
                for n in range(2):
                    k = 2 * pair + n
                    nc.tensor.matmul(sub,
                                     e_sb[k][:, 128 * (j0 + jj):
                                             128 * (j0 + jj + 1)],
                                     Vb_sb[k], start=(n == 0), stop=(n == 1))
            dst = part(j0, 4)
            if pair == 0:
                nc.vector.tensor_copy(dst, pa)
            elif pair < 7:
                nc.vector.tensor_tensor(dst, pa, dst, op=ADD)
            else:                        # final round: bf16 store, host adds
                ob = ob_sb[(j0 // 4) % 4]
                nc.vector.tensor_tensor(ob, pa, dst, op=ADD)
                nc.sync.dma_start(out_r[:, j0:j0 + 4, :], ob)

        # ====== prologue: warmup + bias + first GT block ==================
        with tc.tile_pool(name="ps0", bufs=4, space="PSUM") as ps0:
            def slot0():
                return ps0.tile([128, 512], F32, name="ps0t", tag="ps0t")

            warm = slot0()
            for _ in range(12):
                nc.tensor.matmul(warm, junk[:, 0:128], junk,
                                 start=True, stop=True)
            pt = slot0()
            nc.tensor.matmul(pt[:, 0:D], ones, bv_sb, start=True, stop=True)
            nc.vector.tensor_copy(bvb_sb, pt[:, 0:D])
            for i in range(2):
                gt_group(slot0(), i, 0)

        # ====== main loop: scores + exp + filler/AV per key tile ==========
        with tc.tile_pool(name="psav", bufs=2, space="PSUM") as psav:

            def filler_block(k):
                if k == 0:
                    for v in range(8):
                        pa = psav.tile([128, 1024], F32, name="psavt",
                                       tag="psavt")
                        v_group(pa[:, 0:D], v)
                        if v == 1:
                            gt_group(pa[:, 512:1024], 0, 1)
                        elif v == 3:
                            gt_group(pa[:, 512:1024], 1, 1)
                elif k == 1:
                    for v in range(8, NKT):
                        pa = psav.tile([128, 1024], F32, name="psavt",
                                       tag="psavt")
                        v_group(pa[:, 0:D], v)
                        if v < 12:
                            gt_group(pa[:, 512:1024], (v - 8) % 2,
                                     2 + (v - 8) // 2)
                else:
                    pair, half = k // 2 - 1, k % 2
                    for j0 in range(16 * half, 16 * half + 16, 4):
                        pa = psav.tile([128, 1024], F32, name="psavt",
                                       tag="psavt")
                        emit_av_group(pa, j0, pair)

            with tc.tile_pool(name="psb", bufs=2, space="PSUM") as psb:
                for k in range(NKT):
                    sparts = wpool.tile([128, 4], F32, name="sparts4",
                                        tag="sparts4")
                    for quarter in range(4):
                        s = psb.tile([128, 1024], F32, name="psbt",
                                     tag="psbt")
                        for g in range(2):
                            emit_score_mms(s[:, 512 * g:512 * (g + 1)], k,
                                           1024 * quarter + 512 * g)
                        nc.scalar.activation(
                            e_sb[k][:, 1024 * quarter:1024 * (quarter + 1)],
                            s, EXP, scale=1.0 / 16.0,
                            accum_out=sparts[:, quarter:quarter + 1])
                        if k == 0 and quarter == 1:
                            filler_block(0)  # cover the xT tail-chunk DMA
                    if k != 0:
                        filler_block(k)
                    emit_row_scale(k, sparts)

            # ====== tail: AV pair 7 + partial add + store =================
            with tc.tile_pool(name="psav2", bufs=2, space="PSUM") as psav2:
                for j0 in range(0, NQT, 4):
                    if j0 % 8 == 0:
                        pa = psav.tile([128, 1024], F32, name="psavt",
                                       tag="psavt")
                    else:
                        pa = psav2.tile([128, 1024], F32, name="psav2t",
                                        tag="psav2t")
                    emit_av_group(pa, j0, 7)


def build():
    nc = bacc.Bacc("TRN2", target_bir_lowering=False, debug=False)
    xT = nc.dram_tensor("xT", [D, S], BF, kind="ExternalInput").ap()
    wv2 = nc.dram_tensor("wv2", [D, 2 * D], BF, kind="ExternalInput").ap()
    gbc = nc.dram_tensor("gbc", [D, 1], F32, kind="ExternalInput").ap()
    bvr = nc.dram_tensor("bvr", [1, D], BF, kind="ExternalInput").ap()
    out = nc.dram_tensor("out", [S, D], BF, kind="ExternalOutput").ap()

    with tile.TileContext(nc) as tc:
        _emit(tc, xT, wv2, gbc, bvr, out)
    nc.compile()
    return nc


_NC = None


def _get_nc():
    global _NC
    if _NC is None:
        _NC = build()
    return _NC


def make_in_maps(x, Wq, bq, Wk, bk, Wv, bv):
    # bq cancels under the softmax-over-queries (see module docstring)
    w2T = (np.asarray(Wk, np.float32).T @ np.asarray(Wq, np.float32))
    wv2 = np.ascontiguousarray(
        np.concatenate([w2T, np.asarray(Wv).T], axis=1)).astype(BF16)
    gbc = (np.asarray(Wq, np.float32).T
           @ np.asarray(bk, np.float32)).reshape(D, 1).astype(np.float32)
    bvr = np.asarray(bv).reshape(1, D).astype(BF16)
    in_maps = []
    for core in range(NCORES):
        b, h = divmod(core, 2)
        xTb = np.asarray(x[b]).T.astype(BF16)
        if h:  # rotate so this core's keys are always columns 0:KH
            xTb = np.concatenate([xTb[:, KH:], xTb[:, :KH]], axis=1)
        in_maps.append({
            "xT": np.ascontiguousarray(xTb),
            "wv2": wv2, "gbc": gbc, "bvr": bvr,
        })
    return in_maps


def run(x, Wq, bq, Wk, bk, Wv, bv, trace=False):
    """Run on the 8 cores; returns (full_output, BassKernelResults)."""
    nc = _get_nc()
    in_maps = make_in_maps(x, Wq, bq, Wk, bk, Wv, bv)
    res = run_bass_kernel_spmd(nc, in_maps, core_ids=list(range(NCORES)),
                               trace=trace)
    parts = []
    for core in range(NCORES):
        p = res.results[core]["out"].astype(np.float32)
        if core % 2:  # undo the query rotation
            p = np.concatenate([p[KH:], p[:KH]], axis=0)
        parts.append(p)
    full = np.stack([parts[2 * b] + parts[2 * b + 1] for b in range(B)], axis=0)
    return full.astype(np.float32), res


def kernel(x, Wq, bq, Wk, bk, Wv, bv):
    full, _ = run(x, Wq, bq, Wk, bk, Wv, bv, trace=False)
    return full
